# revision 1
# baseline (speedup 1.0000x reference)
"""Trainium2 Bass kernel for nn_BaconAdditionReasoner (histogram_binning).

Math (per batch row b):
    P1 = soft_perm(W1), P2 = soft_perm(W2)          (host, 10x10)
    l1 = p1 @ P1.T, l2 = p2 @ P2.T                  (device matmul)
    s[i,j] = min(l1[i], l2[j])
    log1m[i,j] = log(1 - s[i,j])  ==  max(u[i], v[j])   where u = log(1-l1), v = log(1-l2)
    logprod[k] = sum_{i+j=k} max(u_i, v_j)
              = sum_{i+j=k} u_i  +  sum_{i+j=k} relu(v_j - u_i)
    y = 1 - exp(logprod);  out = y / sum_k(y)
       with sum_k(y) = 19 - E,  E = sum_k exp(logprod)

Device dataflow (pure data parallel over 8 cores, 32768 rows/core):
  layout: features on partitions, batch on the free dim.  4 "chunks" of 512
  batch rows stacked on 32-aligned partition bands -> one supertile = 2048 rows.
  - L-matmul:  4 concurrent 32x32-tiled matmuls (blockdiag soft-perm weights)
  - ACT:       uv = Ln(1 - l)
  - D-matmul:  4 concurrent row-tiled matmuls expand (v_j - u_i) pairs + pass -u through
  - ACT/DVE:   relu (split by columns between the two engines)
  - A-matmul:  4 concurrent col-tiled matmuls reduce pairs over anti-diagonals (k=i+j)
               and add S_k, giving 2*logprod; zero-pads k=19..31
  - ACT:       e = Exp(0.5 * x)   (junk rows exp(0)=1.0, used below)
  - E-matmul:  E128 = (sum_k e_k) - 19, replicated to each 32-row band
  - DVE:       r = 1/E128 ; out = (e - 1) * r  ==  (1-e)/(19-E)
  - output written k-on-partition as yraw [128, 8192]; host de-interleaves.
"""

import numpy as np

# ---------------------------------------------------------------- constants
B = 262144
NCORES = 8
BC = B // NCORES            # 32768 rows per core
F = 512                     # batch columns per chunk per supertile
CH = 4                      # chunks per supertile (32-aligned partition bands)
ROWS_ST = F * CH            # 2048 rows per supertile
NST = BC // ROWS_ST         # 16 supertiles per core
NCOLS = NST * F             # 8192 columns in pc / yraw

# const (lhsT) column layout inside the wk tensor
WL0, WL1 = 0, 32            # L weights  [20 rows used, 32 cols], replicated per band
WD0, WD1 = 32, 142          # D weights  [20, 110], replicated per band
WA0, WA1 = 142, 174         # A weights  [110, 32]
WE0, WE1 = 174, 302         # E weights  [128, 128]
WKC = 302
KD = 110                    # pair rows (100) + passthrough -u rows (10)

ABS_ACT_COLS = 1024         # columns of |.| done on ScalarE (rest on VectorE)
USE_F32R = True             # float32r (1 cyc/row) for the +-1-coefficient matmuls


def _soft_perm_np(W: np.ndarray) -> np.ndarray:
    W = W.astype(np.float32)
    lo = W.min(axis=1, keepdims=True)
    hi = W.max(axis=1, keepdims=True)
    Wn = (W - lo) / (hi - lo + np.float32(1e-8))
    return Wn / (Wn.sum(axis=1, keepdims=True) + np.float32(1e-8))


def _build_wk(P1n: np.ndarray, P2n: np.ndarray) -> np.ndarray:
    wk = np.zeros((128, WKC), dtype=np.float32)
    # --- L: lhsT[k=e, m=d] = blockdiag(P1n.T, P2n.T), M padded to 32 (zeros)
    for q in range(4):
        r = 32 * q
        wk[r : r + 10, WL0 : WL0 + 10] = P1n.T          # [e, d] = P1n[d, e]
        wk[r + 10 : r + 20, WL0 + 10 : WL0 + 20] = P2n.T
    # --- D: pairs m=10i+j get v_j - u_i ; cols 100..109 pass -u through
    #     (both stay >= 0 after the relu for the max(u,v)=u+relu(v-u) trick)
    d = np.zeros((20, KD), dtype=np.float32)
    for i in range(10):
        for j in range(10):
            d[i, 10 * i + j] = -1.0
            d[10 + j, 10 * i + j] = 1.0
    for e in range(10):
        d[e, 100 + e] = -1.0
    for q in range(4):
        wk[32 * q : 32 * q + 20, WD0:WD1] = d
    # --- A: [110, 32]; rows m<100: +1 at k=i+j ; rows 100+e: -1 for
    #     k in [e, e+9] (those rows hold -u, so -1 gives +u)
    a = np.zeros((KD, 32), dtype=np.float32)
    for i in range(10):
        for j in range(10):
            a[10 * i + j, i + j] = 1.0
    for e in range(10):
        a[100 + e, e : e + 10] = -1.0
    wk[0:KD, WA0:WA1] = a
    # --- E: col p=32g+j <- +1 * e-rows (32g+k, k<19) and -19 * row 32g+19
    #     (that row holds exp(0.5*0)=1.0), so E128[p] = sum_k e_k - 19
    ee = np.zeros((128, 128), dtype=np.float32)
    for g in range(4):
        for j in range(32):
            ee[32 * g : 32 * g + 19, 32 * g + j] = 1.0
            ee[32 * g + 19, 32 * g + j] = -19.0
    wk[0:128, WE0:WE1] = ee
    return wk


def _build_pc(p1c: np.ndarray, p2c: np.ndarray) -> np.ndarray:
    """[BC,10]x2 -> pc [80, NCOLS]: row 20q+e = feature e (u: e<10, v: e>=10)
    of chunk-band q; col F*s+f = batch row ROWS_ST*s + F*q + f."""
    u = p1c.reshape(NST, CH, F, 10).transpose(1, 3, 0, 2).reshape(CH, 10, NCOLS)
    v = p2c.reshape(NST, CH, F, 10).transpose(1, 3, 0, 2).reshape(CH, 10, NCOLS)
    return np.ascontiguousarray(
        np.concatenate([u, v], axis=1).reshape(CH * 20, NCOLS)
    )


def _unpack_yraw(yraw: np.ndarray) -> np.ndarray:
    """yraw [76, NCOLS] -> y [BC, 19]."""
    t = yraw.reshape(4, 19, NST, F).transpose(2, 0, 3, 1)  # [s, g, f, 19]
    return np.ascontiguousarray(t.reshape(BC, 19))


def _patch_act_tables():
    """Force Ln/Exp/Abs to resolve to the single set that has all three
    (natural_log_exp_and_others); the greedy per-function chooser otherwise
    ping-pongs natural_log <-> exp_and_others every supertile (~2.7us/load)."""
    import concourse.bacc as bacc
    import concourse.hw_specs as hw_specs
    from concourse import mybir

    if getattr(bacc, "_act_tables_patched", False):
        return
    orig = bacc.get_activation_tables
    AF = mybir.ActivationFunctionType
    shared = {AF.Ln, AF.Exp, AF.Abs}

    def patched(arch):
        tabs = orig(arch)
        if "natural_log_exp_and_others" in tabs:
            for name, funcs in tabs.items():
                if name != "natural_log_exp_and_others":
                    tabs[name] = set(funcs) - shared
        return tabs

    bacc.get_activation_tables = patched
    bacc._act_tables_patched = True


def build_bass(use_absorbers: bool = False, use_f32r: bool = USE_F32R):
    import concourse.bass as bass
    import concourse.bacc as bacc
    import concourse.tile as tile
    from concourse import mybir
    from concourse.tile import add_dep_helper

    _patch_act_tables()
    f32 = mybir.dt.float32
    f32r = mybir.dt.float32r
    AF = mybir.ActivationFunctionType
    ALU = mybir.AluOpType

    nc = bacc.Bacc("TRN2", target_bir_lowering=False)

    def absorb(deps):
        """PE nop chain, one single-sem wait per producer, so matmuls
        (whose LDWEIGHTS slot fits only one sync wait) start wait-free."""
        if not use_absorbers:
            return None
        last = None
        for d in deps:
            if d is None:
                continue
            n = nc.tensor.nop(nofuse=True)
            add_dep_helper(n.ins, d.ins, sync=True, reason="wait-absorb")
            if last is not None:
                add_dep_helper(n.ins, last.ins, sync=False, reason="absorb-chain")
            last = n
        return last

    def gated(mm, gate):
        if gate is not None:
            add_dep_helper(mm.ins, gate.ins, sync=False, reason="gated")
        return mm
    pc_d = nc.dram_tensor("pc", [80, NCOLS], f32, kind="ExternalInput")
    wk_d = nc.dram_tensor("wk", [128, WKC], f32, kind="ExternalInput")
    y_d = nc.dram_tensor("yraw", [76, NCOLS], f32, kind="ExternalOutput")

    with tile.TileContext(nc) as tc:
        with (
            tc.tile_pool(name="singles", bufs=1) as singles,
            tc.tile_pool(name="pack", bufs=3) as pack_p,
            tc.tile_pool(name="uv", bufs=2) as uv_p,
            tc.tile_pool(name="kt", bufs=2) as kt_p,
            tc.tile_pool(name="ep", bufs=2) as ep_p,
            tc.tile_pool(name="rr", bufs=2) as rr_p,
            tc.tile_pool(name="oo", bufs=3) as oo_p,
            tc.tile_pool(name="psL", bufs=2, space="PSUM") as psL,
            tc.tile_pool(name="psD", bufs=1, space="PSUM") as psD,
            tc.tile_pool(name="psA", bufs=1, space="PSUM") as psA,
            tc.tile_pool(name="psE", bufs=1, space="PSUM") as psE,
        ):
            wk = singles.tile([128, WKC], f32)
            wk_dma = nc.sync.dma_start(wk[:, :], wk_d[:, :])
            if use_f32r:
                # rounded copy: f32r matmul operands must come from a
                # rounding producer (weights are 0/+-1/-19 -> exact)
                wk_r = singles.tile([128, WKC], f32r)
                wk_rnd = nc.vector.tensor_copy(wk_r[:, :], wk[:, :])
            else:
                wk_r, wk_rnd = wk, wk_dma

            log_i = abs_a_i = abs_v_i = exp_i = rcp_i = None
            for s in range(NST):
                off = F * s
                pack = pack_p.tile([128, F], f32)
                dmas = [] if s else [wk_dma]
                for q in range(4):
                    dmas.append(nc.sync.dma_start(
                        pack[32 * q : 32 * q + 20, :],
                        pc_d[20 * q : 20 * q + 20, off : off + F],
                    ))
                # l = blockdiag(P1n, P2n) @ p   (4 concurrent diag tiles)
                gate = absorb(dmas + [log_i])
                lp = psL.tile([128, F], f32)
                for q in range(4):
                    r = 32 * q
                    gated(nc.tensor.matmul(
                        lp[r : r + 32, :],
                        wk[r : r + 20, WL0:WL1],
                        pack[r : r + 20, :],
                        start=True, stop=True,
                        tile_position=(r, r),
                    ), gate)
                # uv = log(1 - l)
                uv = uv_p.tile([128, F], f32r if use_f32r else f32)
                log_i = nc.scalar.activation(
                    uv[:, :], lp[:, :], AF.Ln, bias=1.0, scale=-1.0
                )
                # pair diffs u_i - v_j (+ u,v pass-through)
                gate = absorb([log_i, abs_a_i, abs_v_i, None if s else wk_rnd])
                dp = psD.tile([KD, CH * F], f32)
                for q in range(4):
                    r = 32 * q
                    gated(nc.tensor.matmul(
                        dp[0:KD, q * F : (q + 1) * F],
                        wk_r[r : r + 20, WD0:WD1],
                        uv[r : r + 20, :],
                        start=True, stop=True,
                        tile_position=(r, 0),
                    ), gate)
                # |.| split between ScalarE and VectorE
                kt = kt_p.tile([KD, CH * F], f32r if use_f32r else f32)
                abs_a_i = nc.scalar.activation(
                    kt[:, 0:ABS_ACT_COLS], dp[:, 0:ABS_ACT_COLS], AF.Relu
                )
                abs_v_i = nc.vector.tensor_scalar(
                    kt[:, ABS_ACT_COLS:], dp[:, ABS_ACT_COLS:],
                    0.0, None, op0=ALU.max,
                )
                # anti-diagonal reduce -> 2*logprod (cols k=19..31 zeroed)
                gate = absorb([abs_a_i, abs_v_i])
                ap_ = psA.tile([128, F], f32)
                for g in range(4):
                    # f32r cannot col-tile (ISA); run A in plain f32
                    gated(nc.tensor.matmul(
                        ap_[32 * g : 32 * g + 32, :],
                        wk[0:KD, WA0:WA1],
                        kt[0:KD, g * F : (g + 1) * F].bitcast(f32),
                        start=True, stop=True,
                        tile_position=(0, 32 * g),
                    ), gate)
                # e = exp(logprod); junk rows = exp(0) = 1
                ep = ep_p.tile([128, F], f32r if use_f32r else f32)
                exp_i = nc.scalar.activation(ep[:, :], ap_[:, :], AF.Exp)
                # E128 = sum_k e_k - 19, broadcast to the whole 32-band
                gate = absorb([exp_i, rcp_i])
                e128 = psE.tile([128, F], f32)
                gated(nc.tensor.matmul(
                    e128[:, :], wk_r[0:128, WE0:WE1], ep[:, :],
                    start=True, stop=True
                ), gate)
                rr = rr_p.tile([128, F], f32)
                rcp_i = nc.vector.reciprocal(rr[:, :], e128[:, :])
                oo = oo_p.tile([128, F], f32)
                nc.vector.scalar_tensor_tensor(
                    oo[:, :], ep[:, :].bitcast(f32), 1.0, rr[:, :],
                    op0=ALU.subtract, op1=ALU.mult,
                )
                for g in range(4):
                    nc.sync.dma_start(
                        y_d[19 * g : 19 * g + 19, off : off + F],
                        oo[32 * g : 32 * g + 19, :],
                    )
    nc.compile()
    return nc


_NC_CACHE = None


def kernel(p1, p2, W1, W2):
    global _NC_CACHE
    from concourse.bass_utils import run_bass_kernel_spmd

    P1n = _soft_perm_np(np.asarray(W1))
    P2n = _soft_perm_np(np.asarray(W2))
    wk = _build_wk(P1n, P2n)
    p1 = np.ascontiguousarray(np.asarray(p1, dtype=np.float32))
    p2 = np.ascontiguousarray(np.asarray(p2, dtype=np.float32))

    in_maps = []
    for c in range(NCORES):
        sl = slice(c * BC, (c + 1) * BC)
        in_maps.append({"pc": _build_pc(p1[sl], p2[sl]), "wk": wk})

    if _NC_CACHE is None:
        _NC_CACHE = build_bass()
    res = run_bass_kernel_spmd(_NC_CACHE, in_maps, core_ids=list(range(NCORES)))
    out = np.concatenate(
        [_unpack_yraw(res.results[c]["yraw"]) for c in range(NCORES)], axis=0
    )
    return out



# revision 2
# speedup vs baseline: 2.9184x; 2.9184x over previous
"""Trainium2 Bass kernel for nn_BaconAdditionReasoner (histogram_binning).

Math (per batch row):
    P1 = soft_perm(W1), P2 = soft_perm(W2)           (host, 10x10)
    l1 = p1 @ P1.T, l2 = p2 @ P2.T
    u = log(1-l1), v = log(1-l2)
    logprod[k] = sum_{i+j=k} max(u_i, v_j)
              = sum_{i+j=k} u_i + sum_{i+j=k} relu(v_j - u_i)
    e = exp(logprod);  out_k = (e_k - 1) / (sum_k e_k - 19)

Device dataflow (data parallel over 8 cores, 32768 rows/core):
  Front is feature-major: 4 bands of 20 feature rows at 32-aligned
  partitions (PE tile_position requires 32-aligned moving bases), batch on
  the free dim; 8 supertiles of 4 bands x 1024 cols = 4096 rows.
  - L matmul: one blockdiag [116->116] f16 matmul per supertile
  - Ln (ACT): uv = log(1 - l), f16
  - D matmuls: per band, [20->110] f16 pair-diff expansion (v_j - u_i,
    plus -u passthrough rows)
  - relu: split across ACT / DVE / Pool by column ranges (tunable)
  - A-flip matmuls: per 128-col block, kt [110, 128] is loaded as the
    STATIONARY operand and a tiny [110, 19] +-1 matrix streams as the
    moving operand (19 cycles/block, LDWEIGHTS is free) -> batch-major
    logprod [128 rows, 19 k's] in PSUM
  - Exp (ACT) -> f32, per-row 19-group reduce / recip / (e-1)*r on DVE
  - output dumped partition-major [128, 4864] f16; host de-interleaves.

All HBM I/O and matmul moving operands are f16 (validated on the real
input distribution: max rel err ~2.7e-3 vs the 2e-2 gate).
"""

import numpy as np

# ---------------------------------------------------------------- constants
B = 262144
NCORES = 8
BC = B // NCORES            # 32768 rows per core
F = 1024                    # batch columns per supertile (per band)
NB = 4                      # bands (32-aligned partition offsets)
ROWS_ST = F * NB            # 4096 rows per supertile
NST = BC // ROWS_ST         # 8 supertiles per core
NCOLS = NST * F             # 8192 columns in pc
NBLK = ROWS_ST // 128       # 32 A-flip blocks per supertile
KC = 19 * NBLK              # 608 output cols per supertile
OCOLS = KC * NST            # 4864 output cols

# wk (constants, f16 [128, 256]) column layout
WL0, WL1 = 0, 116           # L blockdiag lhsT [116, 116]
WD0, WD1 = 116, 226         # D pair lhsT [20, 110] replicated per band
WA0, WA1 = 226, 245         # A-flip moving [110, 19]
WKC = 256                   # padded so DMA elem = 512 B

# relu column split per dp tile q: list of (engine, c0, c1)
# engines: "A" = ACT (scalar), "D" = DVE (vector), "P" = Pool (gpsimd)
RELU_SCHEME = [
    [("A", 0, F)],
    [("D", 0, F)],
    [("P", 0, F)],
    [("D", 0, 256), ("P", 256, F)],
]


def _soft_perm_np(W: np.ndarray) -> np.ndarray:
    W = W.astype(np.float32)
    lo = W.min(axis=1, keepdims=True)
    hi = W.max(axis=1, keepdims=True)
    Wn = (W - lo) / (hi - lo + np.float32(1e-8))
    return Wn / (Wn.sum(axis=1, keepdims=True) + np.float32(1e-8))


def _build_wk(P1n: np.ndarray, P2n: np.ndarray) -> np.ndarray:
    wk = np.zeros((128, WKC), dtype=np.float32)
    # --- L: lhsT[32q+d, 32q+e] = PP[e, d], PP = blockdiag(P1n, P2n)
    for q in range(NB):
        r = 32 * q
        wk[r : r + 10, r : r + 10] = P1n.T
        wk[r + 10 : r + 20, r + 10 : r + 20] = P2n.T
    # --- D: [20, 110]: pair col 10i+j gets v_j - u_i; col 100+e gets -u_e
    d = np.zeros((20, 110), dtype=np.float32)
    for i in range(10):
        for j in range(10):
            d[i, 10 * i + j] = -1.0
            d[10 + j, 10 * i + j] = 1.0
    for e in range(10):
        d[e, 100 + e] = -1.0
    for q in range(NB):
        wk[32 * q : 32 * q + 20, WD0:WD1] = d
    # --- A-flip moving [110, 19]: pair rows +1 at k=i+j; passthrough rows
    #     (-u values) -1 for k in [e, e+9]
    a = np.zeros((110, 19), dtype=np.float32)
    for i in range(10):
        for j in range(10):
            a[10 * i + j, i + j] = 1.0
    for e in range(10):
        a[100 + e, e : e + 10] = -1.0
    wk[0:110, WA0:WA1] = a
    return wk.astype(np.float16)


def _build_pc(p1c: np.ndarray, p2c: np.ndarray) -> np.ndarray:
    """[BC,10]x2 -> pc [116, NCOLS] f16: row 32q+e = feature e (u: e<10,
    v: 10<=e<20) of band q; col F*s+f = batch row ROWS_ST*s + F*q + f."""
    pc = np.zeros((116, NCOLS), dtype=np.float16)
    x1 = p1c.reshape(NST, NB, F, 10)    # [s, q, f, d]
    x2 = p2c.reshape(NST, NB, F, 10)
    for q in range(NB):
        pc[32 * q : 32 * q + 10, :] = (
            x1[:, q].transpose(2, 0, 1).reshape(10, NCOLS).astype(np.float16)
        )
        pc[32 * q + 10 : 32 * q + 20, :] = (
            x2[:, q].transpose(2, 0, 1).reshape(10, NCOLS).astype(np.float16)
        )
    return pc


def _unpack_yraw(yraw: np.ndarray) -> np.ndarray:
    """yraw [128, OCOLS] f16 -> y [BC, 19] f32.
    yraw[p, KC*s + 19*b + k] = y[ROWS_ST*s + 128*b + p, k]."""
    t = yraw.reshape(128, NST, NBLK, 19).transpose(1, 2, 0, 3)
    return np.ascontiguousarray(t.reshape(BC, 19).astype(np.float32))


def _patch_act_tables():
    """Force Ln/Exp/Relu to resolve to the single set containing all three
    (natural_log_exp_and_others) so the activation table is loaded once."""
    import concourse.bacc as bacc
    from concourse import mybir

    if getattr(bacc, "_act_tables_patched", False):
        return
    orig = bacc.get_activation_tables
    AF = mybir.ActivationFunctionType
    shared = {AF.Ln, AF.Exp, AF.Relu}

    def patched(arch):
        tabs = orig(arch)
        if "natural_log_exp_and_others" in tabs:
            for name, funcs in tabs.items():
                if name != "natural_log_exp_and_others":
                    tabs[name] = set(funcs) - shared
        return tabs

    bacc.get_activation_tables = patched
    bacc._act_tables_patched = True


def build_bass():
    import concourse.bass as bass
    import concourse.bacc as bacc
    import concourse.tile as tile
    from concourse import mybir

    _patch_act_tables()
    f32 = mybir.dt.float32
    f16 = mybir.dt.float16
    AF = mybir.ActivationFunctionType
    ALU = mybir.AluOpType

    nc = bacc.Bacc("TRN2", target_bir_lowering=False)

    pc_d = nc.dram_tensor("pc", [116, NCOLS], f16, kind="ExternalInput")
    wk_d = nc.dram_tensor("wk", [128, WKC], f16, kind="ExternalInput")
    y_d = nc.dram_tensor("yraw", [128, OCOLS], f16, kind="ExternalOutput")

    with tile.TileContext(nc) as tc:
        with (
            tc.tile_pool(name="singles", bufs=1) as singles,
            tc.tile_pool(name="pcs", bufs=2) as pcs_p,
            tc.tile_pool(name="uv", bufs=2) as uv_p,
            tc.tile_pool(name="kt", bufs=2) as kt_p,
            tc.tile_pool(name="ee", bufs=2) as ee_p,
            tc.tile_pool(name="ss", bufs=2) as ss_p,
            tc.tile_pool(name="rr", bufs=2) as rr_p,
            tc.tile_pool(name="psL", bufs=1, space="PSUM") as psL,
            tc.tile_pool(name="psD", bufs=2, space="PSUM") as psD,
            tc.tile_pool(name="psA", bufs=1, space="PSUM") as psA,
        ):
            wk = singles.tile([128, WKC], f16)
            nc.sync.dma_start(wk[:, :], wk_d[:, :])
            oo = singles.tile([128, OCOLS], f16)

            pcc = None
            for s in range(NST):
                if s % 2 == 0:
                    pcc = pcs_p.tile([116, 2 * F], f16)
                    c0 = (s // 2) * 2 * F
                    nc.sync.dma_start(pcc[:, :], pc_d[:, c0 : c0 + 2 * F])
                off = F * (s % 2)

                # l = blockdiag(P1n, P2n) @ p  (one matmul, f16 moving)
                lp = psL.tile([116, F], f32)
                nc.tensor.matmul(
                    lp[:, :], wk[0:116, WL0:WL1], pcc[0:116, off : off + F],
                    start=True, stop=True,
                )
                # uv = log(1 - l)
                uvt = uv_p.tile([116, F], f16)
                nc.scalar.activation(
                    uvt[:, :], lp[:, :], AF.Ln, bias=1.0, scale=-1.0
                )

                ap_t = psA.tile([128, KC], f32)
                for q in range(NB):
                    r = 32 * q
                    dp = psD.tile([110, F], f32)
                    nc.tensor.matmul(
                        dp[:, :], wk[r : r + 20, WD0:WD1], uvt[r : r + 20, :],
                        start=True, stop=True, tile_position=(r, 0),
                    )
                    kt = kt_p.tile([110, F], f16)
                    for eng, a0, a1 in RELU_SCHEME[q]:
                        if eng == "A":
                            nc.scalar.activation(
                                kt[:, a0:a1], dp[:, a0:a1], AF.Relu
                            )
                        elif eng == "D":
                            nc.vector.tensor_scalar(
                                kt[:, a0:a1], dp[:, a0:a1], 0.0, None,
                                op0=ALU.max,
                            )
                        else:
                            nc.gpsimd.tensor_scalar(
                                kt[:, a0:a1], dp[:, a0:a1], 0.0, None,
                                op0=ALU.max,
                            )
                    # batch-major logprod: kt block stationary, [110,19] moving
                    for b in range(F // 128):
                        blk = (F // 128) * q + b
                        nc.tensor.matmul(
                            ap_t[:, 19 * blk : 19 * blk + 19],
                            kt[0:110, 128 * b : 128 * b + 128],
                            wk[0:110, WA0:WA1],
                            start=True, stop=True,
                        )

                # e = exp(logprod)  (f32: e-1 cancellation needs mantissa)
                e32 = ee_p.tile([128, KC], f32)
                nc.scalar.activation(e32[:, :], ap_t[:, :], AF.Exp)
                ev = e32[:, :].rearrange("p (b k) -> p b k", b=NBLK, k=19)
                s32 = ss_p.tile([128, NBLK], f32)
                nc.vector.tensor_reduce(
                    s32[:, :], ev, axis=mybir.AxisListType.X, op=ALU.add
                )
                sm = ss_p.tile([128, NBLK], f32)
                nc.vector.tensor_scalar(
                    sm[:, :], s32[:, :], -19.0, None, op0=ALU.add
                )
                r32 = rr_p.tile([128, NBLK], f32)
                nc.vector.reciprocal(r32[:, :], sm[:, :])
                # out = (e - 1) * r  ==  (1-e)/(19-sum(e)), f16
                ov = oo[:, KC * s : KC * (s + 1)].rearrange(
                    "p (b k) -> p b k", b=NBLK, k=19
                )
                rb = r32[:, :].unsqueeze(-1).broadcast_to([128, NBLK, 19])
                nc.vector.scalar_tensor_tensor(
                    ov, ev, 1.0, rb, op0=ALU.subtract, op1=ALU.mult
                )
                if s % 2 == 1:
                    o0 = KC * (s - 1)
                    nc.sync.dma_start(
                        y_d[:, o0 : o0 + 2 * KC], oo[:, o0 : o0 + 2 * KC]
                    )
    nc.compile()
    return nc


_NC_CACHE = None


def kernel(p1, p2, W1, W2):
    global _NC_CACHE
    from concourse.bass_utils import run_bass_kernel_spmd

    P1n = _soft_perm_np(np.asarray(W1))
    P2n = _soft_perm_np(np.asarray(W2))
    wk = _build_wk(P1n, P2n)
    p1 = np.ascontiguousarray(np.asarray(p1, dtype=np.float32))
    p2 = np.ascontiguousarray(np.asarray(p2, dtype=np.float32))

    in_maps = []
    for c in range(NCORES):
        sl = slice(c * BC, (c + 1) * BC)
        in_maps.append({"pc": _build_pc(p1[sl], p2[sl]), "wk": wk})

    if _NC_CACHE is None:
        _NC_CACHE = build_bass()
    res = run_bass_kernel_spmd(_NC_CACHE, in_maps, core_ids=list(range(NCORES)))
    out = np.concatenate(
        [_unpack_yraw(res.results[c]["yraw"]) for c in range(NCORES)], axis=0
    )
    return out
